# revision 8
# baseline (speedup 1.0000x reference)
"""Trainium2 Bass kernel for nn_AFM_54022098649750 (dense_mlp).

Reference computation (B=2048, DIM=512, C=64, INTER=128):
    h = relu(bn1(einsum('bdc,cid->bci', x, W1) + b1))
    y = bn2(einsum('bci,cdi->bcd', h, W2) + b2)
    out = sigmoid(transpose(y, (0,2,1)))         # (B, DIM, C)

Strategy:
  * Fold the inference-mode BatchNorms into the conv weights/biases on the
    host (z*s + t == X @ (W*s)^T + (b*s + t)).
  * Branch-parallel sharding: each of the 8 cores owns C_LOC=8 independent
    branches (branch c only consumes x[:, :, c]), so there is no cross-core
    communication and weights are not replicated.
  * Host pre-transposes x to (C, DIM, B) so each branch's activations are
    DMA'd as large fully-contiguous slabs, and the TensorEngine consumes
    them directly: MM1 computes H^T = W1e @ X^T with the DIM contraction on
    partitions, MM2 consumes H^T in place (INTER contraction on partitions).
  * Per (branch, b-tile): 4 accumulating matmuls -> PSUM -> DVE
    relu(psum + b1) -> SBUF -> 4 matmuls -> 4x ScalarE sigmoid(psum + b2)
    into an output slab, DMA'd out as one contiguous transfer per branch.
  * The kernel is DMA-bound: activations stream in/out as bf16 (PSUM
    accumulation and the bias/sigmoid epilogues stay fp32), halving HBM
    traffic.  Set MODE="f32r" for full-fp32 I/O with reduced-precision
    (tf32-like) matmuls instead.
"""

import ml_dtypes
import numpy as np

import concourse.bacc as bacc
import concourse.bass as bass
import concourse.mybir as mybir
import concourse.tile as tile
from concourse.bass_utils import run_bass_kernel_spmd

B, DIM, C, INTER = 2048, 512, 64, 128
EPS = 1e-5
N_CORES = 8
C_LOC = C // N_CORES          # branches per core
KD = DIM // 128               # contraction / output chunks of MM1 / MM2
NB = 512                      # matmul moving free dim (max with fp32 PSUM out)
BT = B // NB                  # b-tiles per branch

MODE = "bf16"                 # "bf16" (half the HBM traffic) or "f32r"

F32 = mybir.dt.float32
AFT = mybir.ActivationFunctionType

_CACHE = {}


def _io_dt():
    return mybir.dt.bfloat16 if MODE == "bf16" else mybir.dt.float32r


def _io_np():
    return ml_dtypes.bfloat16 if MODE == "bf16" else np.float32


def _build():
    """Build + compile the per-core Bass graph (same graph on all cores)."""
    IO = _io_dt()
    nc = bacc.Bacc("TRN2", target_bir_lowering=False, debug=False,
                   num_devices=N_CORES)

    xt = nc.dram_tensor("xt", [C_LOC, DIM, B], IO, kind="ExternalInput").ap()
    w1t = nc.dram_tensor("w1t", [C_LOC, DIM, INTER], IO, kind="ExternalInput").ap()
    w2t = nc.dram_tensor("w2t", [C_LOC, INTER, DIM], IO, kind="ExternalInput").ap()
    bt = nc.dram_tensor("bt", [128, (KD + 1) * C_LOC], F32,
                        kind="ExternalInput").ap()
    out = nc.dram_tensor("out", [C_LOC, KD, 128, B], IO, kind="ExternalOutput").ap()

    with tile.TileContext(nc) as tc:
        with (
            tc.tile_pool(name="xp", bufs=3 * KD) as xp,
            tc.tile_pool(name="op", bufs=3 * KD) as op,
            tc.tile_pool(name="w1p", bufs=4) as w1p,
            tc.tile_pool(name="w2p", bufs=4) as w2p,
            tc.tile_pool(name="hp", bufs=2 * BT) as hp,
            tc.tile_pool(name="bp", bufs=1) as bp,
            tc.tile_pool(name="php", bufs=2, space="PSUM") as php,
            tc.tile_pool(name="pyp", bufs=3, space="PSUM") as pyp,
        ):
            bs = bp.tile([128, (KD + 1) * C_LOC], F32, tag="bs")
            nc.gpsimd.dma_start(bs[:], bt[:])
            b1 = bs[:, 0:C_LOC]
            b2 = bs[:, C_LOC:]

            for c in range(C_LOC):
                # branch 0's weights first on the SP ring so the first
                # matmul isn't gated on the slower SWDGE path; steady
                # state weights go via GpSimd (SWDGE) off both HW rings.
                weng = nc.sync if c == 0 else nc.gpsimd
                w1 = w1p.tile([128, KD * INTER], IO, tag="w1")
                weng.dma_start(
                    w1[:].rearrange("p (k i) -> p k i", k=KD),
                    w1t[c].rearrange("(k p) i -> p k i", p=128),
                )
                w2 = w2p.tile([INTER, DIM], IO, tag="w2")
                weng.dma_start(w2[:], w2t[c])
                # x chunks for branch c: partition = d within chunk k,
                # free = b. One contiguous DMA per chunk.
                xs = []
                for k in range(KD):
                    xk = xp.tile([128, B], IO, tag="xs", name=f"xs{c}_{k}")
                    nc.sync.dma_start(xk[:], xt[c, k * 128:(k + 1) * 128, :])
                    xs.append(xk)

                # phase A: H^T tiles for all b-tiles of this branch
                hs = []
                for t in range(BT):
                    ph = php.tile([INTER, NB], F32, tag="ph")
                    for k in range(KD):
                        nc.tensor.matmul(
                            ph[:],
                            w1[:, k * INTER:(k + 1) * INTER],
                            xs[k][:, t * NB:(t + 1) * NB],
                            start=(k == 0),
                            stop=(k == KD - 1),
                        )
                    h = hp.tile([INTER, NB], IO, tag="h", name=f"h{c}_{t}")
                    if MODE == "bf16":
                        # relu(psum + b1) on the Vector engine (keeps
                        # ScalarE free for the sigmoids)
                        nc.vector.tensor_scalar(
                            h[:], ph[:], b1[:, c:c + 1], 0.0,
                            mybir.AluOpType.add, mybir.AluOpType.max,
                        )
                    else:
                        nc.scalar.activation(h[:], ph[:], AFT.Relu,
                                             bias=b1[:, c:c + 1])
                    hs.append(h)

                # phase B: per output chunk k: 4 matmuls -> 2 wide
                # sigmoids -> chunk DMA out (spreads output traffic)
                for k in range(KD):
                    ok = op.tile([128, B], IO, tag="os", name=f"os{c}_{k}")
                    for tp in range(BT // 2):
                        py = pyp.tile([128, 2 * NB], F32, tag="py")
                        for j in range(2):
                            t = 2 * tp + j
                            nc.tensor.matmul(
                                py[:, j * NB:(j + 1) * NB],
                                w2[:, k * 128:(k + 1) * 128],
                                hs[t][:],
                                start=True,
                                stop=True,
                            )
                        nc.scalar.activation(
                            ok[:, 2 * tp * NB:(2 * tp + 2) * NB],
                            py[:], AFT.Sigmoid,
                            bias=b2[:, k * C_LOC + c: k * C_LOC + c + 1],
                        )
                    nc.scalar.dma_start(out[c, k], ok[:])

    nc.compile()
    return nc


def _prep_in_maps(x, W1, b1, g1, be1, m1, v1, W2, b2, g2, be2, m2, v2):
    """Fold BN, transpose to device layouts, slice per-core shards."""
    io_np = _io_np()
    s1 = (g1 / np.sqrt(v1 + EPS)).astype(np.float32)          # (C, INTER)
    b1e = (b1 * s1 + be1 - m1 * s1).astype(np.float32)        # (C, INTER)
    s2 = (g2 / np.sqrt(v2 + EPS)).astype(np.float32)          # (C, DIM)
    b2e = (b2 * s2 + be2 - m2 * s2).astype(np.float32)        # (C, DIM)

    w1t = np.ascontiguousarray(
        (W1 * s1[:, :, None]).transpose(0, 2, 1)).astype(io_np)  # (C, DIM, INTER)
    w2t = np.ascontiguousarray(
        (W2 * s2[:, :, None]).transpose(0, 2, 1)).astype(io_np)  # (C, INTER, DIM)
    xt = np.ascontiguousarray(x.transpose(2, 1, 0)).astype(io_np)  # (C, DIM, B)
    b1tt = np.ascontiguousarray(b1e.T)                        # (INTER, C)
    # (128, KD, C): bias for output chunk k, partition d_in, branch c
    b2tt = np.ascontiguousarray(
        b2e.reshape(C, KD, 128).transpose(2, 1, 0))

    in_maps = []
    for m in range(N_CORES):
        lo, hi = m * C_LOC, (m + 1) * C_LOC
        in_maps.append({
            "xt": np.ascontiguousarray(xt[lo:hi]),
            "w1t": np.ascontiguousarray(w1t[lo:hi]),
            "w2t": np.ascontiguousarray(w2t[lo:hi]),
            "bt": np.concatenate([
                np.ascontiguousarray(b1tt[:, lo:hi]),
                np.ascontiguousarray(
                    b2tt[:, :, lo:hi]).reshape(128, KD * C_LOC),
            ], axis=1),
        })
    return in_maps


def _unshard(results):
    """(C_LOC, KD, 128, B) per core -> (B, DIM, C)."""
    full = np.empty((B, DIM, C), dtype=np.float32)
    for m in range(N_CORES):
        shard = np.asarray(results[m]["out"]).astype(np.float32)
        # (b, k, d_in, c_loc) -> (B, DIM, C_LOC)
        full[:, :, m * C_LOC:(m + 1) * C_LOC] = (
            shard.transpose(3, 1, 2, 0).reshape(B, DIM, C_LOC))
    return full


def _run(in_maps, trace=False, tmpdir=None):
    if "nc" not in _CACHE:
        _CACHE["nc"] = _build()
    return run_bass_kernel_spmd(
        _CACHE["nc"], in_maps, core_ids=list(range(N_CORES)),
        trace=trace, tmpdir=tmpdir)


def kernel(**inputs):
    in_maps = _prep_in_maps(**inputs)
    res = _run(in_maps)
    return _unshard(res.results)


def kernel_with_profile(tmpdir=None, **inputs):
    """Like kernel() but also returns neuron-profile exec_time_ns."""
    in_maps = _prep_in_maps(**inputs)
    res = _run(in_maps, trace=True, tmpdir=tmpdir)
    return _unshard(res.results), res.exec_time_ns


# revision 9
# speedup vs baseline: 1.0152x; 1.0152x over previous
"""Trainium2 Bass kernel for nn_AFM_54022098649750 (dense_mlp).

Reference computation (B=2048, DIM=512, C=64, INTER=128):
    h = relu(bn1(einsum('bdc,cid->bci', x, W1) + b1))
    y = bn2(einsum('bci,cdi->bcd', h, W2) + b2)
    out = sigmoid(transpose(y, (0,2,1)))         # (B, DIM, C)

Strategy:
  * Fold the inference-mode BatchNorms into the conv weights/biases on the
    host (z*s + t == X @ (W*s)^T + (b*s + t)).
  * Branch-parallel sharding: each of the 8 cores owns C_LOC=8 independent
    branches (branch c only consumes x[:, :, c]), so there is no cross-core
    communication and weights are not replicated.
  * Host pre-transposes x to (C, DIM, B) so each branch's activations are
    DMA'd as large fully-contiguous slabs, and the TensorEngine consumes
    them directly: MM1 computes H^T = W1e @ X^T with the DIM contraction on
    partitions, MM2 consumes H^T in place (INTER contraction on partitions).
  * Per (branch, b-tile): 4 accumulating matmuls -> PSUM -> DVE
    relu(psum + b1) -> SBUF -> 4 matmuls -> 4x ScalarE sigmoid(psum + b2)
    into an output slab, DMA'd out as one contiguous transfer per branch.
  * The kernel is DMA-bound: activations stream in/out as bf16 (PSUM
    accumulation and the bias/sigmoid epilogues stay fp32), halving HBM
    traffic.  Set MODE="f32r" for full-fp32 I/O with reduced-precision
    (tf32-like) matmuls instead.
"""

import ml_dtypes
import numpy as np

import concourse.bacc as bacc
import concourse.bass as bass
import concourse.mybir as mybir
import concourse.tile as tile
from concourse.bass_utils import run_bass_kernel_spmd

B, DIM, C, INTER = 2048, 512, 64, 128
EPS = 1e-5
N_CORES = 8
C_LOC = C // N_CORES          # branches per core
KD = DIM // 128               # contraction / output chunks of MM1 / MM2
NB = 512                      # matmul moving free dim (max with fp32 PSUM out)
BT = B // NB                  # b-tiles per branch

MODE = "bf16"                 # "bf16" (half the HBM traffic) or "f32r"

F32 = mybir.dt.float32
AFT = mybir.ActivationFunctionType

_CACHE = {}


def _io_dt():
    return mybir.dt.bfloat16 if MODE == "bf16" else mybir.dt.float32r


def _io_np():
    return ml_dtypes.bfloat16 if MODE == "bf16" else np.float32


def _build():
    """Build + compile the per-core Bass graph (same graph on all cores)."""
    IO = _io_dt()
    nc = bacc.Bacc("TRN2", target_bir_lowering=False, debug=False,
                   num_devices=N_CORES)

    xt = nc.dram_tensor("xt", [C_LOC, DIM, B], IO, kind="ExternalInput").ap()
    w1t = nc.dram_tensor("w1t", [C_LOC, DIM, INTER], IO, kind="ExternalInput").ap()
    w2t = nc.dram_tensor("w2t", [C_LOC, INTER, DIM], IO, kind="ExternalInput").ap()
    bt = nc.dram_tensor("bt", [128, (KD + 1) * C_LOC], F32,
                        kind="ExternalInput").ap()
    out = nc.dram_tensor("out", [C_LOC, KD, 128, B], IO, kind="ExternalOutput").ap()

    with tile.TileContext(nc) as tc:
        with (
            tc.tile_pool(name="xp", bufs=3 * KD) as xp,
            tc.tile_pool(name="op", bufs=3 * KD) as op,
            tc.tile_pool(name="w1p", bufs=2) as w1p,
            tc.tile_pool(name="w2p", bufs=2) as w2p,
            tc.tile_pool(name="hp", bufs=2 * BT) as hp,
            tc.tile_pool(name="bp", bufs=1) as bp,
            tc.tile_pool(name="php", bufs=2, space="PSUM") as php,
            tc.tile_pool(name="pyp", bufs=3, space="PSUM") as pyp,
        ):
            bs = bp.tile([128, (KD + 1) * C_LOC], F32, tag="bs")
            nc.gpsimd.dma_start(bs[:], bt[:])
            b1 = bs[:, 0:C_LOC]
            b2 = bs[:, C_LOC:]

            for c in range(C_LOC):
                # branch 0's weights first on the SP ring so the first
                # matmul isn't gated on the slower SWDGE path; steady
                # state weights go via GpSimd (SWDGE) off both HW rings.
                weng = nc.sync if c == 0 else nc.gpsimd
                w1 = w1p.tile([128, KD * INTER], IO, tag="w1")
                weng.dma_start(
                    w1[:].rearrange("p (k i) -> p k i", k=KD),
                    w1t[c].rearrange("(k p) i -> p k i", p=128),
                )
                w2 = w2p.tile([INTER, DIM], IO, tag="w2")
                weng.dma_start(w2[:], w2t[c])
                # x chunks for branch c: partition = d within chunk k,
                # free = b. One contiguous DMA per chunk.
                xs = []
                for k in range(KD):
                    xk = xp.tile([128, B], IO, tag="xs", name=f"xs{c}_{k}")
                    nc.sync.dma_start(xk[:], xt[c, k * 128:(k + 1) * 128, :])
                    xs.append(xk)

                # phase A: H^T tiles for all b-tiles of this branch
                hs = []
                for t in range(BT):
                    ph = php.tile([INTER, NB], F32, tag="ph")
                    for k in range(KD):
                        nc.tensor.matmul(
                            ph[:],
                            w1[:, k * INTER:(k + 1) * INTER],
                            xs[k][:, t * NB:(t + 1) * NB],
                            start=(k == 0),
                            stop=(k == KD - 1),
                        )
                    h = hp.tile([INTER, NB], IO, tag="h", name=f"h{c}_{t}")
                    if MODE == "bf16":
                        # relu(psum + b1) on the Vector engine (keeps
                        # ScalarE free for the sigmoids)
                        nc.vector.tensor_scalar(
                            h[:], ph[:], b1[:, c:c + 1], 0.0,
                            mybir.AluOpType.add, mybir.AluOpType.max,
                        )
                    else:
                        nc.scalar.activation(h[:], ph[:], AFT.Relu,
                                             bias=b1[:, c:c + 1])
                    hs.append(h)

                # phase B: per output chunk k: 4 matmuls -> 2 wide
                # sigmoids -> chunk DMA out (spreads output traffic)
                for k in range(KD):
                    ok = op.tile([128, B], IO, tag="os", name=f"os{c}_{k}")
                    for tp in range(BT // 2):
                        py = pyp.tile([128, 2 * NB], F32, tag="py")
                        for j in range(2):
                            t = 2 * tp + j
                            nc.tensor.matmul(
                                py[:, j * NB:(j + 1) * NB],
                                w2[:, k * 128:(k + 1) * 128],
                                hs[t][:],
                                start=True,
                                stop=True,
                            )
                        nc.scalar.activation(
                            ok[:, 2 * tp * NB:(2 * tp + 2) * NB],
                            py[:], AFT.Sigmoid,
                            bias=b2[:, k * C_LOC + c: k * C_LOC + c + 1],
                        )
                    nc.scalar.dma_start(out[c, k], ok[:])

    nc.compile()
    return nc


def _prep_in_maps(x, W1, b1, g1, be1, m1, v1, W2, b2, g2, be2, m2, v2):
    """Fold BN, transpose to device layouts, slice per-core shards."""
    io_np = _io_np()
    s1 = (g1 / np.sqrt(v1 + EPS)).astype(np.float32)          # (C, INTER)
    b1e = (b1 * s1 + be1 - m1 * s1).astype(np.float32)        # (C, INTER)
    s2 = (g2 / np.sqrt(v2 + EPS)).astype(np.float32)          # (C, DIM)
    b2e = (b2 * s2 + be2 - m2 * s2).astype(np.float32)        # (C, DIM)

    w1t = np.ascontiguousarray(
        (W1 * s1[:, :, None]).transpose(0, 2, 1)).astype(io_np)  # (C, DIM, INTER)
    w2t = np.ascontiguousarray(
        (W2 * s2[:, :, None]).transpose(0, 2, 1)).astype(io_np)  # (C, INTER, DIM)
    xt = np.ascontiguousarray(x.transpose(2, 1, 0)).astype(io_np)  # (C, DIM, B)
    b1tt = np.ascontiguousarray(b1e.T)                        # (INTER, C)
    # (128, KD, C): bias for output chunk k, partition d_in, branch c
    b2tt = np.ascontiguousarray(
        b2e.reshape(C, KD, 128).transpose(2, 1, 0))

    in_maps = []
    for m in range(N_CORES):
        lo, hi = m * C_LOC, (m + 1) * C_LOC
        in_maps.append({
            "xt": np.ascontiguousarray(xt[lo:hi]),
            "w1t": np.ascontiguousarray(w1t[lo:hi]),
            "w2t": np.ascontiguousarray(w2t[lo:hi]),
            "bt": np.concatenate([
                np.ascontiguousarray(b1tt[:, lo:hi]),
                np.ascontiguousarray(
                    b2tt[:, :, lo:hi]).reshape(128, KD * C_LOC),
            ], axis=1),
        })
    return in_maps


def _unshard(results):
    """(C_LOC, KD, 128, B) per core -> (B, DIM, C)."""
    full = np.empty((B, DIM, C), dtype=np.float32)
    for m in range(N_CORES):
        shard = np.asarray(results[m]["out"]).astype(np.float32)
        # (b, k, d_in, c_loc) -> (B, DIM, C_LOC)
        full[:, :, m * C_LOC:(m + 1) * C_LOC] = (
            shard.transpose(3, 1, 2, 0).reshape(B, DIM, C_LOC))
    return full


def _run(in_maps, trace=False, tmpdir=None):
    if "nc" not in _CACHE:
        _CACHE["nc"] = _build()
    return run_bass_kernel_spmd(
        _CACHE["nc"], in_maps, core_ids=list(range(N_CORES)),
        trace=trace, tmpdir=tmpdir)


def kernel(**inputs):
    in_maps = _prep_in_maps(**inputs)
    res = _run(in_maps)
    return _unshard(res.results)


def kernel_with_profile(tmpdir=None, **inputs):
    """Like kernel() but also returns neuron-profile exec_time_ns."""
    in_maps = _prep_in_maps(**inputs)
    res = _run(in_maps, trace=True, tmpdir=tmpdir)
    return _unshard(res.results), res.exec_time_ns
